# revision 42
# baseline (speedup 1.0000x reference)
"""MoE layer (top-2 of 8 experts) Trainium2 kernel, expert-parallel on 8 cores.

Strategy
--------
Host: computes the router (logits -> softmax -> top-2) in float64, builds the
per-expert token dispatch (capacity C with zero-weight padding), gathers and
lays out per-core inputs for DMA-friendly access, and scatter-adds the
per-expert partial outputs back into the full output (the "all-to-all
dispatch/combine" of the sharding hint, done host-side since the contract is
full input -> full output).

Device (per core, expert e): y = (gelu(x @ w1 + b1) @ w2 + b2) * w_combine
for the C tokens routed to the core's expert. All GEMM operands are bf16
(fp32 PSUM accumulation): bf16 streams 1 elem/cycle/partition through the PE
(~4x the measured fp32r rate) and halves HBM traffic vs fp32; end-to-end rel
l2 error ~3.4e-3. Capacity C=2176 barely covers the max expert load (2175
for the seed-0 inputs); a host-side exact fallback handles any overflow.

Tiling: tokens processed in chunks (multiples of 128); per chunk GEMM1
produces hid [4096, chunk] bf16 (F on partitions) which stays SBUF-resident,
then GEMM2 contracts over F with chunk/128 concurrent PSUM accumulation
groups so each streamed w2 tile is reused many times. Weights are host-retiled
so every DMA lands with large contiguous per-partition runs.

Engine-queue discipline (keeps the PE at ~98% occupancy): gelu+b1 fused on
ScalarE alone (psum fp32 -> hid bf16); GEMM2 evacuation is a single DVE
scalar_tensor_tensor per psum bank, ot = (psum * wc) + (b2*wc), with the
b2*wc outer products precomputed on DVE off the critical path; y writes go
out via gpsimd SWDGE only; weight loads issue on nc.sync, x/wc on nc.scalar
-- so no engine's FIFO ever queues a data-dependent wait ahead of work the
PE needs. w1 is six-deep double-buffered and 12 ot buffers absorb DMA-queue
latency so PSUM banks recycle without stalling the matmul stream.
"""

import numpy as np
import ml_dtypes

# ---------------------------------------------------------------- constants
B, S, H, F, E, TOP_K = 4, 2048, 1024, 4096, 8, 2
T = B * S
CHUNKS = (768, 768, 640)  # per-chunk token counts (each a multiple of 128)
C = sum(CHUNKS)           # per-expert token capacity (mean load is T*K/E = 2048)
NH = H // 128             # 8 h-blocks
NF = F // 128             # 32 f-tiles
FBLK = 4                  # f-tiles per w1 block (512 f-cols)
NFBLK = NF // FBLK
NQ = 8                    # f-tiles per w2 quad tile
NQUAD = NF // NQ          # 4
GMAX = max(CHUNKS) // 128

BF16 = ml_dtypes.bfloat16

_CACHE = {}


def _ncg_split(chunk):
    """Split a chunk into moving-operand groups (<=512, >=256 when possible,
    each a multiple of 128 so GEMM1 groups stay aligned to token-128 blocks)."""
    if chunk == 640:
        return [384, 256]
    if chunk % 2 == 0 and 256 <= chunk // 2 <= 512:
        return [chunk // 2, chunk // 2]
    out = []
    rem = chunk
    while rem > 512:
        out.append(512)
        rem -= 512
    out.append(rem)
    return out


def _build_nc(loop_r=None, unroll=1, staggered=False):
    """loop_r=None: real kernel (straight-line). loop_r=R: perf variant,
    For_i repeats the body R times (marginal-cost HW timing); `unroll`
    copies of the body per For_i iteration amortize the all-engine
    barrier Tile emits at each loop back-edge (a measurement-loop
    artifact; back-to-back kernel invocations pipeline through it).
    staggered=True uses For_i(staggered_reset=True) instead: 4-stage
    staggered semaphore resets, no per-iteration global barrier."""
    import concourse.mybir as mybir
    import concourse.bass as bass
    from concourse import bacc
    from concourse.tile import TileContext
    from contextlib import ExitStack

    F32 = mybir.dt.float32
    BF = mybir.dt.bfloat16
    AFT = mybir.ActivationFunctionType

    nc = bacc.Bacc(None, target_bir_lowering=False)

    # DRAM tensors (host-prepared layouts; see kernel() below)
    xr_d = nc.dram_tensor("xr", [128, NH * C], BF, kind="ExternalInput")
    w1r_d = nc.dram_tensor("w1r", [NFBLK, 128, NH * FBLK * 128], BF, kind="ExternalInput")
    w2r_d = nc.dram_tensor("w2r", [2, NQUAD, 128, NQ * 512], BF, kind="ExternalInput")
    F8 = mybir.dt.float8e4
    # w2 f-tiles 0-1 in fp8 DoubleRow pair layout: [h5, p, i, n]
    w28_d = nc.dram_tensor("w28", [2, 128, 2, 512], F8, kind="ExternalInput")
    b1_d = nc.dram_tensor("b1", [F], F32, kind="ExternalInput")
    b2_d = nc.dram_tensor("b2", [H], F32, kind="ExternalInput")
    wc_d = nc.dram_tensor("wc", [128, C // 128], F32, kind="ExternalInput")
    y_d = nc.dram_tensor("y", [C, H], F32, kind="ExternalOutput")

    with TileContext(nc) as tc:
        with ExitStack() as stk:
            cpool = stk.enter_context(tc.tile_pool(name="consts", bufs=1))
            xp = stk.enter_context(tc.tile_pool(name="xp", bufs=2))
            wcp = stk.enter_context(tc.tile_pool(name="wcp", bufs=2))
            w1p = stk.enter_context(tc.tile_pool(name="w1p", bufs=5))
            w2p = stk.enter_context(tc.tile_pool(name="w2p", bufs=3))
            plp = stk.enter_context(tc.tile_pool(name="plp", bufs=1))
            hidp = stk.enter_context(tc.tile_pool(name="hidp", bufs=NF - 2))
            hid8p = stk.enter_context(tc.tile_pool(name="hid8p", bufs=2))
            w28p = stk.enter_context(tc.tile_pool(name="w28p", bufs=2))
            outp = stk.enter_context(tc.tile_pool(name="outp", bufs=12))
            b2wcp = stk.enter_context(tc.tile_pool(name="b2wcp", bufs=2 * GMAX))
            ps1p = stk.enter_context(tc.tile_pool(name="ps1", bufs=2, space="PSUM"))
            ps2p = stk.enter_context(tc.tile_pool(name="ps2", bufs=GMAX, space="PSUM"))

            b1t = cpool.tile([128, NF], F32, name="b1t")
            b2bc = cpool.tile([128, H], F32, name="b2bc")
            nc.sync.dma_start(out=b1t, in_=b1_d.rearrange("(t p) -> p t", p=128))
            nc.sync.dma_start(
                out=b2bc,
                in_=b2_d.rearrange("(o n) -> o n", o=1).partition_broadcast(128),
            )

            def body(ci, nm, xpre=None):
                chunk = CHUNKS[ci]
                tok0 = sum(CHUNKS[:ci])       # starting token of this chunk
                G = chunk // 128
                # ---- per-chunk combine weights and x slab
                wcc = wcp.tile([128, G], F32, tag="wcc", name=f"wcc_{nm}")
                nc.scalar.dma_start(out=wcc, in_=wc_d[:, tok0 // 128 : tok0 // 128 + G])
                # b2*wc outer products, precomputed off the evacuation path
                b2wcs = {}
                for h5 in range(2):
                    for g in range(G):
                        bt = b2wcp.tile(
                            [128, 512], F32, tag="b2wc", name=f"b2wc_{nm}_{h5}_{g}"
                        )
                        nc.vector.tensor_scalar_mul(
                            bt, b2bc[:, h5 * 512 : (h5 + 1) * 512], wcc[:, g : g + 1]
                        )
                        b2wcs[(h5, g)] = bt
                if xpre is not None:
                    xbh = xpre     # chunk-0 x slab preloaded by the previous iteration
                else:
                    xbh = []
                    for h in range(NH):
                        xt = xp.tile([128, chunk], BF, tag=f"xb{h}", name=f"xb_{nm}_{h}")
                        nc.scalar.dma_start(
                            out=xt,
                            in_=xr_d[:, NH * tok0 + h * chunk : NH * tok0 + (h + 1) * chunk],
                        )
                        xbh.append(xt)

                # ---- GEMM1: hid[f, c] = gelu(w1.T x + b1), F on partitions
                # f-tiles 0-1 land fp8e4 in a g-major DoubleRow pair tile
                # [128, G*2, 128]: slice [:, 2g:2g+2, :] is a contiguous
                # [128, 2, 128] lhsT for the fp8 MM (scale-free e4m3).
                ncgs = _ncg_split(chunk)
                hid8 = hid8p.tile([128, 2 * G, 128], F8, tag="hid8", name=f"hid8_{nm}")
                hids = {}
                for fb in range(NFBLK):
                    w1b = w1p.tile(
                        [128, NH * FBLK * 128], BF, tag="w1b", name=f"w1b_{nm}_{fb}"
                    )
                    nc.sync.dma_start(out=w1b, in_=w1r_d[fb])
                    for fl in range(FBLK):
                        f128 = fb * FBLK + fl
                        if f128 >= 2:
                            hid_t = hidp.tile(
                                [128, chunk], BF, tag="hid", name=f"hid_{nm}_{f128}"
                            )
                            hids[f128] = hid_t
                        c0 = 0
                        for gi, ncg in enumerate(ncgs):
                            ps = ps1p.tile(
                                [128, ncg], F32, tag="ps1", name=f"ps1_{nm}_{f128}_{gi}"
                            )
                            for h in range(NH):
                                nc.tensor.matmul(
                                    ps,
                                    lhsT=w1b[:, h * FBLK * 128 + fl * 128 : h * FBLK * 128 + (fl + 1) * 128],
                                    rhs=xbh[h][:, c0 : c0 + ncg],
                                    start=(h == 0),
                                    stop=(h == NH - 1),
                                )
                            if f128 < 2:
                                # per-g activation writes into the pair tile
                                for k in range(ncg // 128):
                                    g = c0 // 128 + k
                                    nc.scalar.activation(
                                        hid8[:, 2 * g + f128, :],
                                        ps[:, k * 128 : (k + 1) * 128],
                                        AFT.Gelu,
                                        bias=b1t[:, f128 : f128 + 1],
                                    )
                            else:
                                nc.scalar.activation(
                                    hids[f128][:, c0 : c0 + ncg],
                                    ps,
                                    AFT.Gelu,
                                    bias=b1t[:, f128 : f128 + 1],
                                )
                            c0 += ncg

                # ---- GEMM2: y[c, h] = (hid.T w2 + b2) * wc
                # one fp8 DoubleRow MM (f-tiles 0-1, contraction 256) opens
                # each psum group; tiles 2-31 follow in bf16.
                for h5 in range(2):
                    pss = [
                        ps2p.tile([128, 512], F32, tag="ps2", name=f"ps2_{nm}_{h5}_{g}")
                        for g in range(G)
                    ]
                    w28t = w28p.tile([128, 2, 512], F8, tag="w28", name=f"w28_{nm}_{h5}")
                    nc.sync.dma_start(out=w28t, in_=w28_d[h5])
                    for g in range(G):
                        nc.tensor.matmul(
                            pss[g],
                            lhsT=hid8[:, 2 * g : 2 * g + 2, :],
                            rhs=w28t,
                            start=True,
                            stop=False,
                            perf_mode=mybir.MatmulPerfMode.DoubleRow,
                        )
                    for q in range(NQUAD):
                        w2q = w2p.tile(
                            [128, NQ * 512], BF, tag="w2q", name=f"w2q_{nm}_{h5}_{q}"
                        )
                        nc.sync.dma_start(out=w2q, in_=w2r_d[h5, q])
                        for fl in range(NQ):
                            f128 = q * NQ + fl
                            if f128 < 2:
                                continue
                            for g in range(G):
                                nc.tensor.matmul(
                                    pss[g],
                                    lhsT=hids[f128][:, g * 128 : (g + 1) * 128],
                                    rhs=w2q[:, fl * 512 : (fl + 1) * 512],
                                    start=False,
                                    stop=(f128 == NF - 1),
                                )
                    for g in range(G):
                        ot = outp.tile([128, 512], F32, tag="ot", name=f"ot_{nm}_{h5}_{g}")
                        # ot = (psum * wc) + (b2 * wc) in one DVE op
                        nc.vector.scalar_tensor_tensor(
                            ot,
                            pss[g],
                            wcc[:, g : g + 1],
                            b2wcs[(h5, g)],
                            mybir.AluOpType.mult,
                            mybir.AluOpType.add,
                        )
                        nc.gpsimd.dma_start(
                            out=y_d[
                                tok0 + g * 128 : tok0 + (g + 1) * 128,
                                h5 * 512 : (h5 + 1) * 512,
                            ],
                            in_=ot,
                        )

            if loop_r is None:
                for ci in range(len(CHUNKS)):
                    body(ci, ci)
            else:
                # Software-pipeline the post-barrier fill: the first body's
                # chunk-0 x slab is DMA'd by the previous iteration (pre-loop
                # for iteration 0) and survives the loop barrier in SBUF.
                xpre = [
                    plp.tile([128, CHUNKS[0]], BF, name=f"xpre_{h}")
                    for h in range(NH)
                ]

                def refill():
                    for h in range(NH):
                        nc.scalar.dma_start(
                            out=xpre[h],
                            in_=xr_d[:, h * CHUNKS[0] : (h + 1) * CHUNKS[0]],
                        )

                refill()
                with tc.For_i(0, loop_r, 1, staggered_reset=staggered) as _i:
                    for u in range(unroll):
                        for ci in range(len(CHUNKS)):
                            body(ci, f"{u}_{ci}", xpre=xpre if u == 0 and ci == 0 else None)
                            if u == 0 and ci == 1:
                                refill()  # next iteration's chunk-0 x
    nc.compile()
    return nc


def _get_nc(loop_r=None):
    key = ("nc", loop_r)
    if key not in _CACHE:
        _CACHE[key] = _build_nc(loop_r)
    return _CACHE[key]


# ---------------------------------------------------------------- host side
def _route(x2d, router_w):
    """Float64 mirror of the reference router. Returns per-expert padded
    index lists [E, C] and combine weights [E, C]."""
    logits = x2d.astype(np.float64) @ router_w.astype(np.float64).T  # [T, E]
    m = logits.max(axis=1, keepdims=True)
    p = np.exp(logits - m)
    p /= p.sum(axis=1, keepdims=True)
    # top-2 (ties -> lower index, matching jax.lax.top_k)
    order = np.argsort(-p, axis=1, kind="stable")
    top2 = order[:, :TOP_K]  # [T, 2]
    idx = np.zeros((E, C), np.int64)
    wts = np.zeros((E, C), np.float32)
    counts = np.zeros(E, np.int64)
    overflow = []  # (expert, token_ids, weights) beyond capacity (normally empty)
    sel = np.zeros((T, E), bool)
    np.put_along_axis(sel, top2, True, axis=1)
    for e in range(E):
        tok = np.nonzero(sel[:, e])[0]
        n = len(tok)
        if n > C:
            overflow.append((e, tok[C:], p[tok[C:], e].astype(np.float32)))
            tok = tok[:C]
            n = C
        idx[e, :n] = tok
        wts[e, :n] = p[tok, e].astype(np.float32)
        counts[e] = n
    return idx, wts, counts, overflow


def _prep_core_inputs(x2d, idx_e, wts_e, w1_e, b1_e, w2_e, b2_e):
    xg = x2d[idx_e].astype(BF16)         # [C, H] bf16
    # xr[p, NH*tok0 + h*chunk + c] = xg[tok0 + c, h*128 + p]  per chunk
    xr_parts = []
    tok0 = 0
    for chunk in CHUNKS:
        blk = (
            xg[tok0 : tok0 + chunk]
            .reshape(chunk, NH, 128)
            .transpose(2, 1, 0)
            .reshape(128, NH * chunk)
        )
        xr_parts.append(blk)
        tok0 += chunk
    xr = np.concatenate(xr_parts, axis=1)
    # w1r[fb, p, h*FBLK*128 + fl*128 + m] = w1[h*128 + p, fb*512 + fl*128 + m]
    w1r = (
        w1_e.astype(BF16)
        .reshape(NH, 128, NFBLK, FBLK * 128)
        .transpose(2, 1, 0, 3)
        .reshape(NFBLK, 128, NH * FBLK * 128)
    )
    # w2r[h5, q, p, fl*512 + n] = w2[(q*NQ + fl)*128 + p, h5*512 + n]
    w2r = (
        w2_e.astype(BF16)
        .reshape(NQUAD, NQ, 128, 2, 512)
        .transpose(3, 0, 2, 1, 4)
        .reshape(2, NQUAD, 128, NQ * 512)
    )
    wc = np.ascontiguousarray(wts_e.reshape(C // 128, 128).T)  # [128, C/128]
    # w2 f-tiles 0-1, fp8e4 DoubleRow pair layout [h5, p, i, n]
    w28 = np.ascontiguousarray(
        w2_e[:256]
        .reshape(2, 128, 2, 512)
        .transpose(2, 1, 0, 3)
        .astype(ml_dtypes.float8_e4m3)
    )
    return {
        "xr": np.ascontiguousarray(xr),
        "w1r": np.ascontiguousarray(w1r),
        "w2r": np.ascontiguousarray(w2r),
        "w28": w28,
        "b1": np.ascontiguousarray(b1_e),
        "b2": np.ascontiguousarray(b2_e),
        "wc": wc,
    }


def kernel(hidden_states, router_w, w1, b1, w2, b2):
    from concourse.bass_utils import run_bass_kernel_spmd

    x2d = np.ascontiguousarray(
        np.asarray(hidden_states, dtype=np.float32).reshape(T, H)
    )
    router_w = np.asarray(router_w, dtype=np.float32)
    w1 = np.asarray(w1, dtype=np.float32)
    b1 = np.asarray(b1, dtype=np.float32)
    w2 = np.asarray(w2, dtype=np.float32)
    b2 = np.asarray(b2, dtype=np.float32)

    idx, wts, counts, overflow = _route(x2d, router_w)

    nc = _get_nc()
    in_maps = [
        _prep_core_inputs(x2d, idx[e], wts[e], w1[e], b1[e], w2[e], b2[e])
        for e in range(E)
    ]
    res = run_bass_kernel_spmd(nc, in_maps, core_ids=list(range(E)))

    out = np.zeros((T, H), np.float32)
    for e in range(E):
        n = int(counts[e])
        y = res.results[e]["y"]
        out[idx[e, :n]] += y[:n]
    # capacity-overflow tokens (normally none): host-side exact compute
    if overflow:
        import math
        verf = np.vectorize(math.erf)
        for e, tok, w in overflow:
            hid = x2d[tok] @ w1[e] + b1[e]
            hid = 0.5 * hid * (1.0 + verf(hid / np.sqrt(2.0)))
            out[tok] += (hid @ w2[e] + b2[e]) * w[:, None]
    return out.reshape(B, S, H)


# revision 43
# speedup vs baseline: 1.0352x; 1.0352x over previous
"""MoE layer (top-2 of 8 experts) Trainium2 kernel, expert-parallel on 8 cores.

Strategy
--------
Host: computes the router (logits -> softmax -> top-2) in float64, builds the
per-expert token dispatch (capacity C with zero-weight padding), gathers and
lays out per-core inputs for DMA-friendly access, and scatter-adds the
per-expert partial outputs back into the full output (the "all-to-all
dispatch/combine" of the sharding hint, done host-side since the contract is
full input -> full output).

Device (per core, expert e): y = (gelu(x @ w1 + b1) @ w2 + b2) * w_combine
for the C tokens routed to the core's expert. All GEMM operands are bf16
(fp32 PSUM accumulation): bf16 streams 1 elem/cycle/partition through the PE
(~4x the measured fp32r rate) and halves HBM traffic vs fp32; end-to-end rel
l2 error ~3.4e-3. Capacity C=2176 barely covers the max expert load (2175
for the seed-0 inputs); a host-side exact fallback handles any overflow.

Tiling: tokens processed in chunks (multiples of 128); per chunk GEMM1
produces hid [4096, chunk] bf16 (F on partitions) which stays SBUF-resident,
then GEMM2 contracts over F with chunk/128 concurrent PSUM accumulation
groups so each streamed w2 tile is reused many times. Weights are host-retiled
so every DMA lands with large contiguous per-partition runs.

Engine-queue discipline (keeps the PE at ~98% occupancy): gelu+b1 fused on
ScalarE alone (psum fp32 -> hid bf16); GEMM2 evacuation is a single DVE
scalar_tensor_tensor per psum bank, ot = (psum * wc) + (b2*wc), with the
b2*wc outer products precomputed on DVE off the critical path; y writes go
out via gpsimd SWDGE only; weight loads issue on nc.sync, x/wc on nc.scalar
-- so no engine's FIFO ever queues a data-dependent wait ahead of work the
PE needs. w1 is six-deep double-buffered and 12 ot buffers absorb DMA-queue
latency so PSUM banks recycle without stalling the matmul stream.
"""

import numpy as np
import ml_dtypes

# ---------------------------------------------------------------- constants
B, S, H, F, E, TOP_K = 4, 2048, 1024, 4096, 8, 2
T = B * S
CHUNKS = (768, 768, 640)  # per-chunk token counts (each a multiple of 128)
C = sum(CHUNKS)           # per-expert token capacity (mean load is T*K/E = 2048)
NH = H // 128             # 8 h-blocks
NF = F // 128             # 32 f-tiles
FBLK = 4                  # f-tiles per w1 block (512 f-cols)
NFBLK = NF // FBLK
NQ = 8                    # f-tiles per w2 quad tile
NQUAD = NF // NQ          # 4
GMAX = max(CHUNKS) // 128

BF16 = ml_dtypes.bfloat16

_CACHE = {}


def _ncg_split(chunk):
    """Split a chunk into moving-operand groups (<=512, >=256 when possible)."""
    if chunk % 2 == 0 and 256 <= chunk // 2 <= 512:
        return [chunk // 2, chunk // 2]
    out = []
    rem = chunk
    while rem > 512:
        out.append(512)
        rem -= 512
    out.append(rem)
    return out


def _build_nc(loop_r=None, unroll=1, staggered=False):
    """loop_r=None: real kernel (straight-line). loop_r=R: perf variant,
    For_i repeats the body R times (marginal-cost HW timing); `unroll`
    copies of the body per For_i iteration amortize the all-engine
    barrier Tile emits at each loop back-edge (a measurement-loop
    artifact; back-to-back kernel invocations pipeline through it).
    staggered=True uses For_i(staggered_reset=True) instead: 4-stage
    staggered semaphore resets, no per-iteration global barrier."""
    import concourse.mybir as mybir
    import concourse.bass as bass
    from concourse import bacc
    from concourse.tile import TileContext
    from contextlib import ExitStack

    F32 = mybir.dt.float32
    BF = mybir.dt.bfloat16
    AFT = mybir.ActivationFunctionType

    nc = bacc.Bacc(None, target_bir_lowering=False)

    # DRAM tensors (host-prepared layouts; see kernel() below)
    xr_d = nc.dram_tensor("xr", [128, NH * C], BF, kind="ExternalInput")
    w1r_d = nc.dram_tensor("w1r", [NFBLK, 128, NH * FBLK * 128], BF, kind="ExternalInput")
    w2r_d = nc.dram_tensor("w2r", [2, NQUAD, 128, NQ * 512], BF, kind="ExternalInput")
    b1_d = nc.dram_tensor("b1", [F], F32, kind="ExternalInput")
    b2_d = nc.dram_tensor("b2", [H], F32, kind="ExternalInput")
    wc_d = nc.dram_tensor("wc", [128, C // 128], F32, kind="ExternalInput")
    y_d = nc.dram_tensor("y", [C, H], F32, kind="ExternalOutput")

    with TileContext(nc) as tc:
        with ExitStack() as stk:
            cpool = stk.enter_context(tc.tile_pool(name="consts", bufs=1))
            xp = stk.enter_context(tc.tile_pool(name="xp", bufs=2))
            wcp = stk.enter_context(tc.tile_pool(name="wcp", bufs=2))
            w1p = stk.enter_context(tc.tile_pool(name="w1p", bufs=5))
            w2p = stk.enter_context(tc.tile_pool(name="w2p", bufs=3))
            plp = stk.enter_context(tc.tile_pool(name="plp", bufs=1))
            hidp = stk.enter_context(tc.tile_pool(name="hidp", bufs=NF))
            outp = stk.enter_context(tc.tile_pool(name="outp", bufs=12))
            b2wcp = stk.enter_context(tc.tile_pool(name="b2wcp", bufs=2 * GMAX))
            ps1p = stk.enter_context(tc.tile_pool(name="ps1", bufs=2, space="PSUM"))
            ps2p = stk.enter_context(tc.tile_pool(name="ps2", bufs=GMAX, space="PSUM"))

            b1t = cpool.tile([128, NF], F32, name="b1t")
            b2bc = cpool.tile([128, H], F32, name="b2bc")
            nc.sync.dma_start(out=b1t, in_=b1_d.rearrange("(t p) -> p t", p=128))
            nc.sync.dma_start(
                out=b2bc,
                in_=b2_d.rearrange("(o n) -> o n", o=1).partition_broadcast(128),
            )

            def body(ci, nm, xpre=None):
                chunk = CHUNKS[ci]
                tok0 = sum(CHUNKS[:ci])       # starting token of this chunk
                G = chunk // 128
                # ---- per-chunk combine weights and x slab
                wcc = wcp.tile([128, G], F32, tag="wcc", name=f"wcc_{nm}")
                nc.scalar.dma_start(out=wcc, in_=wc_d[:, tok0 // 128 : tok0 // 128 + G])
                # b2*wc outer products, precomputed off the evacuation path
                b2wcs = {}
                for h5 in range(2):
                    for g in range(G):
                        bt = b2wcp.tile(
                            [128, 512], F32, tag="b2wc", name=f"b2wc_{nm}_{h5}_{g}"
                        )
                        nc.vector.tensor_scalar_mul(
                            bt, b2bc[:, h5 * 512 : (h5 + 1) * 512], wcc[:, g : g + 1]
                        )
                        b2wcs[(h5, g)] = bt
                if xpre is not None:
                    xbh = xpre     # chunk-0 x slab preloaded by the previous iteration
                else:
                    xbh = []
                    for h in range(NH):
                        xt = xp.tile([128, chunk], BF, tag=f"xb{h}", name=f"xb_{nm}_{h}")
                        nc.scalar.dma_start(
                            out=xt,
                            in_=xr_d[:, NH * tok0 + h * chunk : NH * tok0 + (h + 1) * chunk],
                        )
                        xbh.append(xt)

                # ---- GEMM1: hid[f, c] = gelu(w1.T x + b1), F on partitions
                ncgs = _ncg_split(chunk)
                hids = []
                for fb in range(NFBLK):
                    w1b = w1p.tile(
                        [128, NH * FBLK * 128], BF, tag="w1b", name=f"w1b_{nm}_{fb}"
                    )
                    nc.sync.dma_start(out=w1b, in_=w1r_d[fb])
                    for fl in range(FBLK):
                        f128 = fb * FBLK + fl
                        hid_t = hidp.tile(
                            [128, chunk], BF, tag="hid", name=f"hid_{nm}_{f128}"
                        )
                        c0 = 0
                        for gi, ncg in enumerate(ncgs):
                            ps = ps1p.tile(
                                [128, ncg], F32, tag="ps1", name=f"ps1_{nm}_{f128}_{gi}"
                            )
                            for h in range(NH):
                                nc.tensor.matmul(
                                    ps,
                                    lhsT=w1b[:, h * FBLK * 128 + fl * 128 : h * FBLK * 128 + (fl + 1) * 128],
                                    rhs=xbh[h][:, c0 : c0 + ncg],
                                    start=(h == 0),
                                    stop=(h == NH - 1),
                                )
                            nc.scalar.activation(
                                hid_t[:, c0 : c0 + ncg],
                                ps,
                                AFT.Gelu,
                                bias=b1t[:, f128 : f128 + 1],
                            )
                            c0 += ncg
                        hids.append(hid_t)

                # ---- GEMM2: y[c, h] = (hid.T w2 + b2) * wc
                for h5 in range(2):
                    pss = [
                        ps2p.tile([128, 512], F32, tag="ps2", name=f"ps2_{nm}_{h5}_{g}")
                        for g in range(G)
                    ]
                    for q in range(NQUAD):
                        w2q = w2p.tile(
                            [128, NQ * 512], BF, tag="w2q", name=f"w2q_{nm}_{h5}_{q}"
                        )
                        nc.sync.dma_start(out=w2q, in_=w2r_d[h5, q])
                        for fl in range(NQ):
                            f128 = q * NQ + fl
                            for g in range(G):
                                nc.tensor.matmul(
                                    pss[g],
                                    lhsT=hids[f128][:, g * 128 : (g + 1) * 128],
                                    rhs=w2q[:, fl * 512 : (fl + 1) * 512],
                                    start=(f128 == 0),
                                    stop=(f128 == NF - 1),
                                )
                    for g in range(G):
                        ot = outp.tile([128, 512], F32, tag="ot", name=f"ot_{nm}_{h5}_{g}")
                        # ot = (psum * wc) + (b2 * wc) in one DVE op
                        nc.vector.scalar_tensor_tensor(
                            ot,
                            pss[g],
                            wcc[:, g : g + 1],
                            b2wcs[(h5, g)],
                            mybir.AluOpType.mult,
                            mybir.AluOpType.add,
                        )
                        nc.gpsimd.dma_start(
                            out=y_d[
                                tok0 + g * 128 : tok0 + (g + 1) * 128,
                                h5 * 512 : (h5 + 1) * 512,
                            ],
                            in_=ot,
                        )

            if loop_r is None:
                for ci in range(len(CHUNKS)):
                    body(ci, ci)
            else:
                # Software-pipeline the post-barrier fill: the first body's
                # chunk-0 x slab is DMA'd by the previous iteration (pre-loop
                # for iteration 0) and survives the loop barrier in SBUF.
                xpre = [
                    plp.tile([128, CHUNKS[0]], BF, name=f"xpre_{h}")
                    for h in range(NH)
                ]

                def refill():
                    for h in range(NH):
                        nc.scalar.dma_start(
                            out=xpre[h],
                            in_=xr_d[:, h * CHUNKS[0] : (h + 1) * CHUNKS[0]],
                        )

                refill()
                with tc.For_i(0, loop_r, 1, staggered_reset=staggered) as _i:
                    for u in range(unroll):
                        for ci in range(len(CHUNKS)):
                            body(ci, f"{u}_{ci}", xpre=xpre if u == 0 and ci == 0 else None)
                            if u == 0 and ci == 1:
                                refill()  # next iteration's chunk-0 x
    nc.compile()
    return nc


def _get_nc(loop_r=None):
    key = ("nc", loop_r)
    if key not in _CACHE:
        _CACHE[key] = _build_nc(loop_r)
    return _CACHE[key]


# ---------------------------------------------------------------- host side
def _route(x2d, router_w):
    """Float64 mirror of the reference router. Returns per-expert padded
    index lists [E, C] and combine weights [E, C]."""
    logits = x2d.astype(np.float64) @ router_w.astype(np.float64).T  # [T, E]
    m = logits.max(axis=1, keepdims=True)
    p = np.exp(logits - m)
    p /= p.sum(axis=1, keepdims=True)
    # top-2 (ties -> lower index, matching jax.lax.top_k)
    order = np.argsort(-p, axis=1, kind="stable")
    top2 = order[:, :TOP_K]  # [T, 2]
    idx = np.zeros((E, C), np.int64)
    wts = np.zeros((E, C), np.float32)
    counts = np.zeros(E, np.int64)
    overflow = []  # (expert, token_ids, weights) beyond capacity (normally empty)
    sel = np.zeros((T, E), bool)
    np.put_along_axis(sel, top2, True, axis=1)
    for e in range(E):
        tok = np.nonzero(sel[:, e])[0]
        n = len(tok)
        if n > C:
            overflow.append((e, tok[C:], p[tok[C:], e].astype(np.float32)))
            tok = tok[:C]
            n = C
        idx[e, :n] = tok
        wts[e, :n] = p[tok, e].astype(np.float32)
        counts[e] = n
    return idx, wts, counts, overflow


def _prep_core_inputs(x2d, idx_e, wts_e, w1_e, b1_e, w2_e, b2_e):
    xg = x2d[idx_e].astype(BF16)         # [C, H] bf16
    # xr[p, NH*tok0 + h*chunk + c] = xg[tok0 + c, h*128 + p]  per chunk
    xr_parts = []
    tok0 = 0
    for chunk in CHUNKS:
        blk = (
            xg[tok0 : tok0 + chunk]
            .reshape(chunk, NH, 128)
            .transpose(2, 1, 0)
            .reshape(128, NH * chunk)
        )
        xr_parts.append(blk)
        tok0 += chunk
    xr = np.concatenate(xr_parts, axis=1)
    # w1r[fb, p, h*FBLK*128 + fl*128 + m] = w1[h*128 + p, fb*512 + fl*128 + m]
    w1r = (
        w1_e.astype(BF16)
        .reshape(NH, 128, NFBLK, FBLK * 128)
        .transpose(2, 1, 0, 3)
        .reshape(NFBLK, 128, NH * FBLK * 128)
    )
    # w2r[h5, q, p, fl*512 + n] = w2[(q*NQ + fl)*128 + p, h5*512 + n]
    w2r = (
        w2_e.astype(BF16)
        .reshape(NQUAD, NQ, 128, 2, 512)
        .transpose(3, 0, 2, 1, 4)
        .reshape(2, NQUAD, 128, NQ * 512)
    )
    wc = np.ascontiguousarray(wts_e.reshape(C // 128, 128).T)  # [128, C/128]
    return {
        "xr": np.ascontiguousarray(xr),
        "w1r": np.ascontiguousarray(w1r),
        "w2r": np.ascontiguousarray(w2r),
        "b1": np.ascontiguousarray(b1_e),
        "b2": np.ascontiguousarray(b2_e),
        "wc": wc,
    }


def kernel(hidden_states, router_w, w1, b1, w2, b2):
    from concourse.bass_utils import run_bass_kernel_spmd

    x2d = np.ascontiguousarray(
        np.asarray(hidden_states, dtype=np.float32).reshape(T, H)
    )
    router_w = np.asarray(router_w, dtype=np.float32)
    w1 = np.asarray(w1, dtype=np.float32)
    b1 = np.asarray(b1, dtype=np.float32)
    w2 = np.asarray(w2, dtype=np.float32)
    b2 = np.asarray(b2, dtype=np.float32)

    idx, wts, counts, overflow = _route(x2d, router_w)

    nc = _get_nc()
    in_maps = [
        _prep_core_inputs(x2d, idx[e], wts[e], w1[e], b1[e], w2[e], b2[e])
        for e in range(E)
    ]
    res = run_bass_kernel_spmd(nc, in_maps, core_ids=list(range(E)))

    out = np.zeros((T, H), np.float32)
    for e in range(E):
        n = int(counts[e])
        y = res.results[e]["y"]
        out[idx[e, :n]] += y[:n]
    # capacity-overflow tokens (normally none): host-side exact compute
    if overflow:
        import math
        verf = np.vectorize(math.erf)
        for e, tok, w in overflow:
            hid = x2d[tok] @ w1[e] + b1[e]
            hid = 0.5 * hid * (1.0 + verf(hid / np.sqrt(2.0)))
            out[tok] += (hid @ w2[e] + b2[e]) * w[:, None]
    return out.reshape(B, S, H)
